# revision 5
# baseline (speedup 1.0000x reference)
"""Trainium2 Bass kernel for nn_Conv2D_80796924772741.

Depthwise (grouped, F=64) 3x3 valid conv over [F, 514, 514, 4] int8 with
per-channel int8 weights + int32 bias, followed by exact fixed-point requant
  res = (acc * 19920 + 2^21) >> 22 ;  out = clip(res - 5, -128, 127) int8
(reduced_mantissa 19920 = 1245 * 16 -> res = (acc*1245 + 2^17) >> 18).

Sharding: F=64 split across 8 NeuronCores (8 channels each), embarrassingly
parallel.

Per-core compute, per (channel, H-window) group ([M<=124 rows, 2048 cols]):
 - PE: conv via Toeplitz-band stationary matmuls over H-windows
   (contraction = input rows; all 3 H-taps in the band diagonals; 3
   matmuls per 512-col chunk for the 3 W-taps, W-shift = +4n free-dim
   offset). Bias b rides two all-ones rhs partitions. PSUM = acc+b exact.
 - ACT1: hif = fp16(ps * 2^-8 - 0.2490234375) = (hi/2) exactly, where
   hi = floor((acc+b)/128): value grid is 0.5, |offset| < 0.25, no ties,
   |hi/2| <= 584 so fp16 halves are exact.
 - PE:   ps  += (-256*I) @ hif  -> lo = (acc+b) mod 128 in PSUM
 - ACT2: q16  = int16(ps * 9.7265625 - 0.498046875)  [= floor(lo*1245/128)]
 - DVE:  S32  = hif * 2490 + q16   (scalar_tensor_tensor; exact < 2^21)
 - DVE:  out  = int8(S32 * 2^-11 - 4.999755859375)
   [RNE -> floor(S/2^11 + 1/2) - 5 = res - 5; int8 SATURATION == clip]
All intermediates exact in fp32; conversion semantics (RNE+saturate on both
ACT and DVE, fp32-internal ALU) verified on hardware. Bit-exact vs the
int64 reference.
"""

import numpy as np
import ml_dtypes

F_PER_CORE = 8
H_IN = 514
W_IN = 514
D = 4
H_OUT = 512
WD_OUT = 2048  # 512 * 4
FREE_IN = W_IN * D  # 2056
N_CHUNK = 512
N_CORES = 8

# H windows: output rows per window (partition-limited: K = M + 4 <= 128)
WINDOWS = [(0, 124), (124, 124), (248, 124), (372, 124), (496, 16)]


def _build_lhsT(w_core: np.ndarray, b_core: np.ndarray) -> np.ndarray:
    """[128, 8*3*124] bf16 stationary: per (channel, w-tap) a Toeplitz band.

    Layout column block (f*3 + n)*124 : +124  holds T_n for channel f.
    T_n[2 + i + m, i] = w[f, m, n]  (rows 2.. are conv data partitions)
    T_0[0, i] = 8*floor(b/8) ; T_0[1, i] = b mod 8  (bias rows, multiplied
    by all-ones rhs partitions 0/1; both parts bf16-exact).
    """
    out = np.zeros((128, F_PER_CORE * 3 * 124), dtype=np.float32)
    for f in range(F_PER_CORE):
        b_f = int(b_core[f])
        bh = b_f >> 3  # floor division
        bl = b_f - 8 * bh
        for n in range(3):
            base = (f * 3 + n) * 124
            if n == 0:
                out[0, base : base + 124] = float(8 * bh)
                out[1, base : base + 124] = float(bl)
            for m in range(3):
                wv = float(int(w_core[f, m, n, 0]))
                idx = np.arange(124)
                out[2 + idx + m, base + idx] = wv
    return out.astype(ml_dtypes.bfloat16)


_PROGRAM_CACHE = {}


def _build_program():
    import concourse.bass as bass
    import concourse.tile as tile
    from concourse import bacc, mybir

    nc = bacc.Bacc(
        "TRN2", target_bir_lowering=False, debug=False, num_devices=N_CORES
    )
    dt = mybir.dt
    Alu = mybir.AluOpType
    Act = mybir.ActivationFunctionType

    x_d = nc.dram_tensor(
        "x", [F_PER_CORE, H_IN, FREE_IN], dt.int8, kind="ExternalInput"
    ).ap()
    lhsT_d = nc.dram_tensor(
        "lhsT", [128, F_PER_CORE * 3 * 124], dt.bfloat16, kind="ExternalInput"
    ).ap()
    id_d = nc.dram_tensor("id4", [124, 124], dt.float16, kind="ExternalInput").ap()
    ones_d = nc.dram_tensor("ones2", [2, FREE_IN], dt.bfloat16, kind="ExternalInput").ap()
    y_d = nc.dram_tensor(
        "y", [F_PER_CORE, H_OUT, WD_OUT], dt.int8, kind="ExternalOutput"
    ).ap()

    groups = [(f, r0, m_r) for f in range(F_PER_CORE) for (r0, m_r) in WINDOWS]

    with tile.TileContext(nc) as tc:
        with (
            tc.tile_pool(name="const", bufs=1) as const_pool,
            tc.tile_pool(name="xin", bufs=3) as x_pool,
            tc.tile_pool(name="psum", bufs=2, space="PSUM") as psum_pool,
            tc.tile_pool(name="hif", bufs=2) as hif_pool,
            tc.tile_pool(name="q16", bufs=2) as q_pool,
            tc.tile_pool(name="s32", bufs=2) as s_pool,
            tc.tile_pool(name="out8", bufs=3) as o_pool,
        ):
            lhsT_t = const_pool.tile([128, F_PER_CORE * 3 * 124], dt.bfloat16)
            nc.sync.dma_start(lhsT_t[:], lhsT_d[:])
            id_t = const_pool.tile([124, 124], dt.float16)
            nc.sync.dma_start(id_t[:], id_d[:])

            def fixup(f, r0, m_r, ps, hif):
                # fixup: ps += -256 * (hi/2)  -> lo in PSUM
                for c in range(4):
                    nc.tensor.matmul(
                        ps[0:m_r, c * N_CHUNK : (c + 1) * N_CHUNK],
                        id_t[0:m_r, 0:m_r],
                        hif[0:m_r, c * N_CHUNK : (c + 1) * N_CHUNK],
                        start=False,
                        stop=True,
                        skip_group_check=True,
                    )

            def phase2(f, r0, m_r, ps, hif):
                # q16 = floor(lo * 1245/128)
                q16 = q_pool.tile([124, WD_OUT], dt.int16)
                nc.scalar.activation(
                    q16[0:m_r, :], ps[0:m_r, :], Act.Copy,
                    bias=-0.498046875, scale=9.7265625,
                )
                # S = (hi/2)*2490 + q  (exact, < 2^21)
                s32 = s_pool.tile([124, WD_OUT], dt.int32)
                nc.vector.scalar_tensor_tensor(
                    s32[0:m_r, :], hif[0:m_r, :], 2490.0, q16[0:m_r, :],
                    Alu.mult, Alu.add,
                )
                # out = sat8(RNE(S*2^-11 - 4.999755859375)) = clip(res-5)
                o8 = o_pool.tile([124, WD_OUT], dt.int8)
                nc.vector.tensor_scalar(
                    o8[0:m_r, :], s32[0:m_r, :], 0.00048828125,
                    -4.999755859375, Alu.mult, Alu.add,
                )
                nc.sync.dma_start(y_d[f, r0 : r0 + m_r, :], o8[0:m_r, :])

            prev = None
            for (f, r0, m_r) in groups:
                k_r = m_r + 4  # 2 ones rows + m_r + 2 data rows
                xt = x_pool.tile([128, FREE_IN], dt.bfloat16)
                # ones rows (bias partitions)
                nc.sync.dma_start(xt[0:2, :], ones_d[:])
                # data rows with int8 -> bf16 cast (SWDGE)
                nc.gpsimd.dma_start(
                    xt[2 : 2 + m_r + 2, :], x_d[f, r0 : r0 + m_r + 2, :]
                )
                ps = psum_pool.tile([124, WD_OUT], dt.float32)

                def conv_banks(cs, ce):
                    for n in range(3):
                        base = (f * 3 + n) * 124
                        for c in range(cs, ce):
                            nc.tensor.matmul(
                                ps[0:m_r, c * N_CHUNK : (c + 1) * N_CHUNK],
                                lhsT_t[0:k_r, base : base + m_r],
                                xt[0:k_r, c * N_CHUNK + 4 * n : c * N_CHUNK + 4 * n + N_CHUNK],
                                start=(n == 0),
                                stop=False,
                                skip_group_check=True,
                            )

                # interleave prev group's fixup between conv halves so the
                # PE FIFO loop (conv -> fixup -> ACT2 -> psum free) overlaps
                conv_banks(0, 2)
                if prev is not None:
                    fixup(*prev)
                conv_banks(2, 4)
                if prev is not None:
                    phase2(*prev)
                # hif = (hi/2) in fp16, hi = floor((acc+b)/128); exact:
                # RNE-to-fp16-halves of (acc+b)/256 - 0.2490234375
                hif = hif_pool.tile([124, WD_OUT], dt.float16)
                nc.scalar.activation(
                    hif[0:m_r, :], ps[0:m_r, :], Act.Copy,
                    bias=-0.2490234375, scale=0.00390625,
                )
                prev = (f, r0, m_r, ps, hif)
            fixup(*prev)
            phase2(*prev)

    nc.compile()
    return nc


def _make_in_maps(x: np.ndarray, w: np.ndarray, b: np.ndarray) -> list:
    id4 = (-256.0 * np.eye(124, dtype=np.float32)).astype(np.float16)
    ones2 = np.ones((2, FREE_IN), dtype=np.float32).astype(ml_dtypes.bfloat16)
    in_maps = []
    for core in range(N_CORES):
        lo = core * F_PER_CORE
        hi = lo + F_PER_CORE
        x_shard = np.ascontiguousarray(x[lo:hi]).reshape(F_PER_CORE, H_IN, FREE_IN)
        lhsT = _build_lhsT(w[lo:hi], b[lo:hi])
        in_maps.append({"x": x_shard, "lhsT": lhsT, "id4": id4, "ones2": ones2})
    return in_maps


def kernel(x: np.ndarray, w: np.ndarray, b: np.ndarray) -> np.ndarray:
    """x: int8 [64, 514, 514, 4]; w: int8 [64, 3, 3, 1]; b: int32 [64].

    Returns int8 [64, 512, 512, 4].
    """
    from concourse.bass_utils import run_bass_kernel_spmd

    if "nc" not in _PROGRAM_CACHE:
        _PROGRAM_CACHE["nc"] = _build_program()
    nc = _PROGRAM_CACHE["nc"]

    F = x.shape[0]
    assert F == N_CORES * F_PER_CORE

    in_maps = _make_in_maps(x, w, b)
    res = run_bass_kernel_spmd(nc, in_maps, core_ids=list(range(N_CORES)))

    out = np.empty((F, H_OUT, 512, D), dtype=np.int8)
    for core in range(N_CORES):
        lo = core * F_PER_CORE
        y = res.results[core]["y"]  # [8, 512, 2048] int8
        out[lo : lo + F_PER_CORE] = y.reshape(F_PER_CORE, H_OUT, 512, D)
    return out


# revision 9
# speedup vs baseline: 1.0720x; 1.0720x over previous
"""Trainium2 Bass kernel for nn_Conv2D_80796924772741.

Depthwise (grouped, F=64) 3x3 valid conv over [F, 514, 514, 4] int8 with
per-channel int8 weights + int32 bias, followed by exact fixed-point requant
  res = (acc * 19920 + 2^21) >> 22 ;  out = clip(res - 5, -128, 127) int8
(reduced_mantissa 19920 = 1245 * 16 -> res = (acc*1245 + 2^17) >> 18).

Sharding: F=64 split across 8 NeuronCores (8 channels each), embarrassingly
parallel.

Per-core compute, per (channel, H-window) group ([M<=124 rows, 2048 cols];
the last 16 output rows are packed 4-chunks-into-partitions as a [64, 512]
"strip" group so they cost 512-wide ops instead of 4 full-width passes):
 - PE: conv via Toeplitz-band stationary matmuls over H-windows
   (contraction = input rows; all 3 H-taps in the band diagonals; one
   matmul per 512-col chunk per W-tap, W-shift = +4n free-dim offset).
   Bias b rides two all-ones rhs partitions. PSUM = acc+b exact (fp32).
 - ACT1: hi16 = int16(ps * 2^-7 - 0.498046875)  [RNE+sat int conversion
   == floor((acc+b)/128); value grid 1/128 -> no ties]
 - DVE:  hif  = fp16(hi16)    [exact, |hi| <= 1168]
 - PE:   ps  += (-128*I) @ hif  -> lo = (acc+b) mod 128 in PSUM
 - ACT2: q16  = int16(ps * 9.7265625 - 0.498046875)  [= floor(lo*1245/128)]
 - DVE:  S32  = hi16 * 1245 + q16   (scalar_tensor_tensor; exact < 2^21)
 - DVE:  out  = int8(S32 * 2^-11 - 4.999755859375)
   [RNE -> floor(S/2^11 + 1/2) - 5 = res - 5; int8 SATURATION == clip]
All intermediates exact in fp32; conversion semantics (RNE+saturate on both
ACT and DVE, fp32-internal ALU) verified on hardware. Bit-exact vs the
int64 reference.
"""

import numpy as np
import ml_dtypes

F_PER_CORE = 8
H_IN = 514
W_IN = 514
D = 4
H_OUT = 512
WD_OUT = 2048  # 512 * 4
FREE_IN = W_IN * D  # 2056
N_CHUNK = 512
N_CORES = 8

# Full H windows (M=124); rows 496..511 are handled by the packed strip.
FULL_WINDOWS = [(0, 124), (124, 124), (248, 124), (372, 124)]
STRIP_R0 = 496
STRIP_M = 16  # output rows per chunk block
STRIP_KB = 20  # partitions per chunk block: 2 ones + 16+2 data rows


def _build_lhsT(w_core: np.ndarray, b_core: np.ndarray) -> np.ndarray:
    """[128, 8*3*124] bf16 stationary: per (channel, w-tap) a Toeplitz band.

    Layout column block (f*3 + n)*124 : +124  holds T_n for channel f.
    T_n[2 + i + m, i] = w[f, m, n]  (rows 2.. are conv data partitions)
    T_0[0, i] = 8*floor(b/8) ; T_0[1, i] = b mod 8  (bias rows, multiplied
    by all-ones rhs partitions 0/1; both parts bf16-exact).
    """
    out = np.zeros((128, F_PER_CORE * 3 * 124), dtype=np.float32)
    for f in range(F_PER_CORE):
        b_f = int(b_core[f])
        bh = b_f >> 3  # floor division
        bl = b_f - 8 * bh
        for n in range(3):
            base = (f * 3 + n) * 124
            if n == 0:
                out[0, base : base + 124] = float(8 * bh)
                out[1, base : base + 124] = float(bl)
            for m in range(3):
                wv = float(int(w_core[f, m, n, 0]))
                idx = np.arange(124)
                out[2 + idx + m, base + idx] = wv
    return out.astype(ml_dtypes.bfloat16)


def _build_lhsT2(w_core: np.ndarray, b_core: np.ndarray) -> np.ndarray:
    """[80, 8*3*64] bf16 strip stationaries, block-diagonal per chunk.

    Chunk block c occupies partitions 20c..20c+19 (2 ones rows + 18 data
    rows) and psum rows 16c..16c+15. Column block (f*3+n)*64 holds the
    tap-n stationary for channel f covering all 4 chunks.
    """
    out = np.zeros((80, F_PER_CORE * 3 * 64), dtype=np.float32)
    for f in range(F_PER_CORE):
        b_f = int(b_core[f])
        bh = b_f >> 3
        bl = b_f - 8 * bh
        for n in range(3):
            base = (f * 3 + n) * 64
            for c in range(4):
                col0 = base + 16 * c
                row0 = 20 * c
                if n == 0:
                    out[row0 + 0, col0 : col0 + 16] = float(8 * bh)
                    out[row0 + 1, col0 : col0 + 16] = float(bl)
                for m in range(3):
                    wv = float(int(w_core[f, m, n, 0]))
                    idx = np.arange(16)
                    out[row0 + 2 + idx + m, col0 + idx] = wv
    return out.astype(ml_dtypes.bfloat16)


_PROGRAM_CACHE = {}


def _build_program():
    import concourse.bass as bass
    import concourse.tile as tile
    from concourse import bacc, mybir

    nc = bacc.Bacc(
        "TRN2", target_bir_lowering=False, debug=False, num_devices=N_CORES
    )
    dt = mybir.dt
    Alu = mybir.AluOpType
    Act = mybir.ActivationFunctionType

    x_d = nc.dram_tensor(
        "x", [F_PER_CORE, H_IN, FREE_IN], dt.int8, kind="ExternalInput"
    ).ap()
    lhsT_d = nc.dram_tensor(
        "lhsT", [128, F_PER_CORE * 3 * 124], dt.bfloat16, kind="ExternalInput"
    ).ap()
    lhsT2_d = nc.dram_tensor(
        "lhsT2", [80, F_PER_CORE * 3 * 64], dt.bfloat16, kind="ExternalInput"
    ).ap()
    id_d = nc.dram_tensor("id4", [124, 124], dt.float16, kind="ExternalInput").ap()
    ones_d = nc.dram_tensor("ones2", [2, FREE_IN], dt.bfloat16, kind="ExternalInput").ap()
    y_d = nc.dram_tensor(
        "y", [F_PER_CORE, H_OUT, WD_OUT], dt.int8, kind="ExternalOutput"
    ).ap()

    groups = []
    for f in range(F_PER_CORE):
        for (r0, m_r) in FULL_WINDOWS:
            groups.append(("full", f, r0, m_r))
        groups.append(("strip", f, STRIP_R0, STRIP_M))

    with tile.TileContext(nc) as tc:
        with (
            tc.tile_pool(name="const", bufs=1) as const_pool,
            tc.tile_pool(name="xin", bufs=3) as x_pool,
            tc.tile_pool(name="psum", bufs=2, space="PSUM") as psum_pool,
            tc.tile_pool(name="hi16", bufs=2) as hi_pool,
            tc.tile_pool(name="hif", bufs=2) as hif_pool,
            tc.tile_pool(name="q16", bufs=2) as q_pool,
            tc.tile_pool(name="s32", bufs=2) as s_pool,
            tc.tile_pool(name="out8", bufs=3) as o_pool,
        ):
            lhsT_t = const_pool.tile([128, F_PER_CORE * 3 * 124], dt.bfloat16)
            nc.sync.dma_start(lhsT_t[:], lhsT_d[:])
            lhsT2_t = const_pool.tile([80, F_PER_CORE * 3 * 64], dt.bfloat16)
            nc.sync.dma_start(lhsT2_t[:], lhsT2_d[:])
            id_t = const_pool.tile([124, 124], dt.float16)
            nc.sync.dma_start(id_t[:], id_d[:])

            def fixup(prev):
                kind, f, r0, m_r, ps, hi16, hif = prev
                if kind == "full":
                    for c in range(4):
                        nc.tensor.matmul(
                            ps[0:m_r, c * N_CHUNK : (c + 1) * N_CHUNK],
                            id_t[0:m_r, 0:m_r],
                            hif[0:m_r, c * N_CHUNK : (c + 1) * N_CHUNK],
                            start=False,
                            stop=True,
                            skip_group_check=True,
                        )
                else:
                    nc.tensor.matmul(
                        ps[0:64, 0:N_CHUNK],
                        id_t[0:64, 0:64],
                        hif[0:64, 0:N_CHUNK],
                        start=False,
                        stop=True,
                        skip_group_check=True,
                    )

            def phase2(prev):
                kind, f, r0, m_r, ps, hi16, hif = prev
                rows = m_r if kind == "full" else 64
                fd = WD_OUT if kind == "full" else N_CHUNK
                # q16 = floor(lo * 1245/128)
                q16 = q_pool.tile([124, WD_OUT], dt.int16)
                nc.scalar.activation(
                    q16[0:rows, 0:fd], ps[0:rows, 0:fd], Act.Copy,
                    bias=-0.498046875, scale=9.7265625,
                )
                # S = hi*1245 + q  (exact, < 2^21)
                s32 = s_pool.tile([124, WD_OUT], dt.int32)
                nc.vector.scalar_tensor_tensor(
                    s32[0:rows, 0:fd], hi16[0:rows, 0:fd], 1245.0,
                    q16[0:rows, 0:fd], Alu.mult, Alu.add,
                )
                # out = sat8(RNE(S*2^-11 - 4.999755859375)) = clip(res-5)
                o8 = o_pool.tile([124, WD_OUT], dt.int8)
                nc.vector.tensor_scalar(
                    o8[0:rows, 0:fd], s32[0:rows, 0:fd], 0.00048828125,
                    -4.999755859375, Alu.mult, Alu.add,
                )
                if kind == "full":
                    nc.sync.dma_start(y_d[f, r0 : r0 + m_r, :], o8[0:m_r, :])
                else:
                    for c in range(4):
                        nc.sync.dma_start(
                            y_d[f, r0 : r0 + STRIP_M,
                                c * N_CHUNK : (c + 1) * N_CHUNK],
                            o8[16 * c : 16 * c + 16, 0:N_CHUNK],
                        )

            prev = None
            for (kind, f, r0, m_r) in groups:
                if kind == "full":
                    k_r = m_r + 4  # 2 ones rows + m_r + 2 data rows
                    xt = x_pool.tile([128, FREE_IN], dt.bfloat16)
                    nc.sync.dma_start(xt[0:2, :], ones_d[:])
                    nc.gpsimd.dma_start(
                        xt[2 : 2 + m_r + 2, :], x_d[f, r0 : r0 + m_r + 2, :]
                    )
                    ps = psum_pool.tile([124, WD_OUT], dt.float32)
                    for n in range(3):
                        base = (f * 3 + n) * 124
                        for c in range(4):
                            nc.tensor.matmul(
                                ps[0:m_r, c * N_CHUNK : (c + 1) * N_CHUNK],
                                lhsT_t[0:k_r, base : base + m_r],
                                xt[0:k_r, c * N_CHUNK + 4 * n : c * N_CHUNK + 4 * n + N_CHUNK],
                                start=(n == 0),
                                stop=False,
                                skip_group_check=True,
                            )
                    rows, fd = m_r, WD_OUT
                else:
                    xt = x_pool.tile([128, FREE_IN], dt.bfloat16)
                    for c in range(4):
                        p0 = STRIP_KB * c
                        nc.sync.dma_start(xt[p0 : p0 + 2, 0:520], ones_d[:, 0:520])
                        nc.gpsimd.dma_start(
                            xt[p0 + 2 : p0 + STRIP_KB, 0:520],
                            x_d[f, r0 : r0 + 18, c * N_CHUNK : c * N_CHUNK + 520],
                        )
                    ps = psum_pool.tile([124, WD_OUT], dt.float32)
                    for n in range(3):
                        base = (f * 3 + n) * 64
                        nc.tensor.matmul(
                            ps[0:64, 0:N_CHUNK],
                            lhsT2_t[0:80, base : base + 64],
                            xt[0:80, 4 * n : 4 * n + N_CHUNK],
                            start=(n == 0),
                            stop=False,
                            skip_group_check=True,
                        )
                    rows, fd = 64, N_CHUNK

                if prev is not None:
                    fixup(prev)
                    phase2(prev)
                # hi16 = floor((acc+b)/128)  via RNE(x - 0.498046875)
                hi16 = hi_pool.tile([124, WD_OUT], dt.int16)
                nc.scalar.activation(
                    hi16[0:rows, 0:fd], ps[0:rows, 0:fd], Act.Copy,
                    bias=-0.498046875, scale=0.0078125,
                )
                hif = hif_pool.tile([124, WD_OUT], dt.float16)
                nc.vector.tensor_scalar(
                    hif[0:rows, 0:fd], hi16[0:rows, 0:fd], 0, None, Alu.add
                )
                prev = (kind, f, r0, m_r, ps, hi16, hif)
            fixup(prev)
            phase2(prev)

    nc.compile()
    return nc


def _make_in_maps(x: np.ndarray, w: np.ndarray, b: np.ndarray) -> list:
    id4 = (-128.0 * np.eye(124, dtype=np.float32)).astype(np.float16)
    ones2 = np.ones((2, FREE_IN), dtype=np.float32).astype(ml_dtypes.bfloat16)
    in_maps = []
    for core in range(N_CORES):
        lo = core * F_PER_CORE
        hi = lo + F_PER_CORE
        x_shard = np.ascontiguousarray(x[lo:hi]).reshape(F_PER_CORE, H_IN, FREE_IN)
        lhsT = _build_lhsT(w[lo:hi], b[lo:hi])
        lhsT2 = _build_lhsT2(w[lo:hi], b[lo:hi])
        in_maps.append(
            {"x": x_shard, "lhsT": lhsT, "lhsT2": lhsT2, "id4": id4, "ones2": ones2}
        )
    return in_maps


def kernel(x: np.ndarray, w: np.ndarray, b: np.ndarray) -> np.ndarray:
    """x: int8 [64, 514, 514, 4]; w: int8 [64, 3, 3, 1]; b: int32 [64].

    Returns int8 [64, 512, 512, 4].
    """
    from concourse.bass_utils import run_bass_kernel_spmd

    if "nc" not in _PROGRAM_CACHE:
        _PROGRAM_CACHE["nc"] = _build_program()
    nc = _PROGRAM_CACHE["nc"]

    F = x.shape[0]
    assert F == N_CORES * F_PER_CORE

    in_maps = _make_in_maps(x, w, b)
    res = run_bass_kernel_spmd(nc, in_maps, core_ids=list(range(N_CORES)))

    out = np.empty((F, H_OUT, 512, D), dtype=np.int8)
    for core in range(N_CORES):
        lo = core * F_PER_CORE
        y = res.results[core]["y"]  # [8, 512, 2048] int8
        out[lo : lo + F_PER_CORE] = y.reshape(F_PER_CORE, H_OUT, 512, D)
    return out
